# revision 1
# baseline (speedup 1.0000x reference)
"""Trainium2 Bass kernel for nn_MixedIGAB (2-layer IGAB dense-transformer block).

Sharding: 8 cores = (batch b = core//4) x (H-slab s = core%4, 32 rows each).
Halo replication (up to +-6 rows, host-padded) avoids neighbor exchange; one
small AllReduce per layer (per-batch groups of 4 cores) handles the global
channel-attention gram matrix and the q/k L2 norms.

Layout: channel-major [C partitions, flat rows*130] with zero guard columns
at w=0 and w=129 so the 3x3 depthwise convs are flat-shift
scalar_tensor_tensor taps. fp16 operands + fp16 residual stream, fp32 PSUM
accumulation and fp32 attention/LN statistics.
"""
import sys, os

sys.path.insert(0, "/opt/trn_rl_repo")
if "/root/.axon_site" not in sys.path:
    sys.path.append("/root/.axon_site")

import numpy as np
import ml_dtypes

import concourse.bass as bass
import concourse.bacc as bacc
import concourse.mybir as mybir
from concourse import bass_utils
from concourse import tile

F32 = mybir.dt.float32
F32R = mybir.dt.float32r
F16 = mybir.dt.float16
AF = mybir.ActivationFunctionType
ALU = mybir.AluOpType

B, C, H, W = 2, 256, 128, 128
HEADS, D = 8, 32
LYR, FC = 2, 1024
WG = W + 2          # guarded width 130
HALO = 6
HB = 32 + 2 * HALO  # 44 buffer rows
NB = HB * WG        # flat elems per channel (5720)
N_CORES = 8
GROUPS = [[0, 1, 2, 3], [4, 5, 6, 7]]
EPS = 1e-5
P = 128

# per-layer row extents in buffer coords [lo, hi)
EXT = [
    dict(rv=(0, 44), rq=(6, 38), rp1=(1, 43), ry=(2, 42), rdw=(3, 41)),
    dict(rv=(3, 41), rq=(6, 38), rp1=(4, 40), ry=(5, 39), rdw=(6, 38)),
]
FF_CH = 10          # ff chunk rows
POS_CH = 14         # pos-conv accumulator chunk rows
TAPS = [(dy, dx) for dy in (-1, 0, 1) for dx in (-1, 0, 1)]
FFDW_DVE_MT = (0, 1, 2, 3)   # ffdw channel-tiles on DVE (rest on PE)

DBG = [s for s in os.environ.get("KDBG", "").split(",") if s]


def _ntiles(total, step):
    out, o = [], 0
    while o < total:
        n = min(step, total - o)
        out.append((o, n))
        o += n
    return out


def _r3(ap_flat):
    return ap_flat.rearrange("p (r w) -> p r w", w=WG)


def _tap_src(in_fl, ins_fl, t, a, n):
    dy, dx = TAPS[t]
    s0 = (a + dy) * WG
    if dx == 0:
        return in_fl[:, s0:s0 + n]
    if dx == -1:
        return ins_fl[:, s0:s0 + n]
    return ins_fl[:, s0 + 2:s0 + 2 + n]


def _conv9(nc, pool, tag, out_ap, in_fl, ins_fl, wsc, a, b):
    # out_ap may be None -> result lands in a pool tile; returns the AP
    """9-tap 3x3 depthwise conv on guarded flat layout, as DVE
    tensor_scalar products (4x mode) + tensor_tensor tree adds (2x mode).

    out rows [a, b) in in_fl's local row coords; out_ap is the destination
    AP [128, (b-a)*WG]. in_fl must cover rows [a-1, b+1); ins_fl[j] =
    in_fl[j-1] valid for j in [(a-1)*WG, (b+1)*WG + 2)."""
    n = (b - a) * WG

    def ts(t):
        pr = pool.tile([P, n], F16, tag=tag + "p", name=tag + "p", bufs=2)
        nc.vector.tensor_scalar_mul(pr[:], _tap_src(in_fl, ins_fl, t, a, n),
                                    wsc[:, t:t + 1])
        return pr

    def add(x, y, dst=None):
        if dst is None:
            dst = pool.tile([P, n], F16, tag=tag + "s", name=tag + "s",
                            bufs=2)[:]
        nc.vector.tensor_tensor(dst, x, y, ALU.add)
        return dst

    r = add(ts(0)[:], ts(1)[:])
    for t in range(2, 8):
        r = add(r, ts(t)[:])
    return add(r, ts(8)[:], dst=out_ap)


def _conv9_pe(nc, psum_ap, diag, in_fl, a, off, n):
    """9 diagonal-matmul taps accumulating into psum_ap [128, n].
    diag: list of 9 [128,128] f16 diag(w) tiles. Reads in_fl rows from
    (a+dy)*WG + dx + off (misaligned reads are fine on the PE)."""
    for t in range(9):
        dy, dx = TAPS[t]
        s0 = (a + dy) * WG + dx + off
        nc.tensor.matmul(psum_ap, diag[t][:], in_fl[:, s0:s0 + n],
                         start=(t == 0), stop=(t == 8))


def _shift1(nc, dst_fl, src_fl, n):
    """dst[j] = src[j-1] for j in [1, n); dst[0] stays garbage (only ever
    read into guard outputs). dst must have room for n+2 elems; the +2 tail
    is zeroed (taps read it into guard outputs)."""
    nc.vector.tensor_copy(dst_fl[:, 1:n], src_fl[:, 0:n - 1])
    nc.any.memset(dst_fl[:, n:n + 2], 0.0)


def _mask_rows(nc, flat_ap, rmask, lo, hi):
    """Multiply rows of flat_ap that may lie outside the global image
    (buffer rows < HALO or >= HB - HALO) by the per-core row mask."""
    for r in list(range(lo, min(HALO, hi))) + list(range(max(HB - HALO, lo), hi)):
        nc.vector.tensor_scalar_mul(flat_ap[:, r * WG:(r + 1) * WG],
                                    flat_ap[:, r * WG:(r + 1) * WG],
                                    rmask[:, r:r + 1])


def _zero_guards(nc, flat_ap, a, b):
    t3 = _r3(flat_ap)
    nc.any.memset(t3[:, a:b, 0:1], 0.0)
    nc.any.memset(t3[:, a:b, WG - 1:WG], 0.0)


def build():
    nc = bacc.Bacc("TRN2", target_bir_lowering=False, debug=False,
                   num_devices=N_CORES)

    def din(name, shape, dt):
        return nc.dram_tensor(name, list(shape), dt, kind="ExternalInput")

    xh_d = din("xh", (2, P, NB), F16)
    il_d = din("il", (2, P, NB), F16)
    rm_d = din("rmask", (P, HB), F32)
    wqk_d = din("wqk", (LYR, 2, P, 512), F16)
    wv_d = din("wv", (LYR, 2, 2, P, P), F16)
    wpr_d = din("wpr", (LYR, 2, 2, P, P), F16)
    f1t_d = din("f1t", (LYR, 2, 8, P, P), F16)
    f3t_d = din("f3t", (LYR, 8, 2, P, P), F16)
    posw_d = din("posw", (LYR, 2, 2, P, 9), F32)
    dww_d = din("dww", (LYR, 8, P, 9), F32)
    dwdg_d = din("dwdg", (LYR, 8, 9, P, P), F16)
    psdg_d = din("psdg", (LYR, 2, 2, 9, P, P), F16)
    lng_d = din("lng", (LYR, 2, P, 1), F32)
    b1_d = din("b1", (LYR, 8, P, 1), F32)
    bpj_d = din("bpj", (LYR, 2, P, 1), F32)
    resc_d = din("resc", (LYR, 2, P, 1), F32)

    out_d = nc.dram_tensor("out", [2, P, 32, W], F32, kind="ExternalOutput")
    dbg_d = {}

    def dbg(name, src_ap, shape, dt=F16):
        if name not in DBG:
            return
        t = nc.dram_tensor("dbg_" + name, list(shape), dt, kind="ExternalOutput")
        dbg_d[name] = t
        nc.sync.dma_start(t.ap(), src_ap)

    with tile.TileContext(nc) as tc:
        with tc.tile_pool(name="persist", bufs=1) as pp, \
             tc.tile_pool(name="dstage", bufs=1, space="DRAM") as dp:

            xa = [pp.tile([P, NB], F16, tag=f"xa{i}", name=f"xa{i}") for i in range(2)]
            xb = [pp.tile([P, NB], F16, tag=f"xb{i}", name=f"xb{i}") for i in range(2)]
            attnT = pp.tile([P, 2, 32], F16, tag="attnT", name="attnT")
            ones_h = pp.tile([P, P], F16, tag="ones_h", name="ones_h")
            nc.any.memset(ones_h[:], 1.0)
            ones_r = pp.tile([P, P], F32R, tag="ones_r", name="ones_r")
            nc.any.memset(ones_r[:].bitcast(F32), 1.0)
            rmask = pp.tile([P, HB], F32, tag="rmask", name="rmask")
            nc.sync.dma_start(rmask[:], rm_d.ap())
            eps_t = pp.tile([P, 1], F32, tag="eps_t", name="eps_t")
            nc.any.memset(eps_t[:], EPS)
            cinv_t = pp.tile([P, 1], F32, tag="cinv_t", name="cinv_t")
            nc.any.memset(cinv_t[:], 1.0 / C)
            for i in range(2):
                nc.sync.dma_start(xa[i][:], xh_d.ap()[i])

            cc_in = [dp.tile([256, 258], F32, tag=f"ccin{l}", name=f"ccin{l}") for l in range(LYR)]
            cc_out = [dp.tile([256, 258], F32, tag=f"ccout{l}", name=f"ccout{l}") for l in range(LYR)]

            for l in range(LYR):
                with tc.tile_pool(name=f"wp{l}", bufs=1) as wp:
                    def wt(dram_ap, shape, tag):
                        t = wp.tile(list(shape), dram_ap.dtype, tag=tag, name=tag)
                        nc.sync.dma_start(t[:], dram_ap)
                        return t

                    w = dict(
                        wqk=[wt(wqk_d.ap()[l, kt], [P, 512], f"wqk{kt}")
                             for kt in range(2)],
                        wv=[[wt(wv_d.ap()[l, kt, mt], [P, P], f"wv{kt}{mt}")
                             for mt in range(2)] for kt in range(2)],
                    )

                    def wload(phase, l=l, w=w, wt=wt):
                        if phase == "pos" and "posw" not in w:
                            w["posw"] = [[wt(posw_d.ap()[l, cv, ct], [P, 9], f"pw{cv}{ct}")
                                          for ct in range(2)] for cv in range(2)]
                            w["posdiag"] = [[[wt(psdg_d.ap()[l, cv, ct, t], [P, P], f"pg{cv}{ct}{t}")
                                              for t in range(9)] for ct in range(2)]
                                            for cv in range(2)]
                            w["resc"] = [wt(resc_d.ap()[l, ct], [P, 1], f"rs{ct}")
                                         for ct in range(2)]
                        if phase == "oy" and "wpr" not in w:
                            w["wpr"] = [[wt(wpr_d.ap()[l, kt, mt], [P, P], f"wpr{kt}{mt}")
                                         for mt in range(2)] for kt in range(2)]
                            w["bpj"] = [wt(bpj_d.ap()[l, ct], [P, 1], f"bpj{ct}")
                                        for ct in range(2)]
                        if phase == "ff" and "f1t" not in w:
                            w["f1t"] = [[wt(f1t_d.ap()[l, kt, mt], [P, P], f"f1t{kt}{mt}")
                                         for mt in range(8)] for kt in range(2)]
                            w["f3t"] = [[wt(f3t_d.ap()[l, kt, mt], [P, P], f"f3t{kt}{mt}")
                                         for mt in range(2)] for kt in range(8)]
                            w["dww"] = [wt(dww_d.ap()[l, mt], [P, 9], f"dw{mt}")
                                        for mt in range(8)]
                            w["dwdiag"] = [[wt(dwdg_d.ap()[l, mt, t], [P, P], f"dg{mt}{t}")
                                            for t in range(9)] for mt in range(8)]
                            w["lng"] = [wt(lng_d.ap()[l, ct], [P, 1], f"lng{ct}")
                                        for ct in range(2)]
                            w["b1"] = [wt(b1_d.ap()[l, mt], [P, 1], f"b1{mt}")
                                       for mt in range(8)]

                    _layer(nc, tc, l, w, wload, xa, xb, attnT, il_d, ones_h, ones_r,
                           eps_t, cinv_t, rmask,
                           cc_in[l], cc_out[l],
                           out_d if l == LYR - 1 else None, dbg)

    nc.compile()
    return nc, dbg_d


def _layer(nc, tc, l, w, wload, xa, xb, attnT, il_d, ones_h, ones_r, eps_t, cinv_t, rmask, cc_in, cc_out,
           out_d, dbg_raw):
    def dbg(name, src_ap, shape, dt=F16):
        dbg_raw(f"{name}.{l}", src_ap, shape, dt)

    ext = EXT[l]
    rv0, rv1 = ext["rv"]; rq0, rq1 = ext["rq"]
    rp10, rp11 = ext["rp1"]; ry0, ry1 = ext["ry"]; rdw0, rdw1 = ext["rdw"]

    with tc.tile_pool(name=f"mid{l}", bufs=1) as mp:
        v_sb = [mp.tile([P, NB + 2], F16, tag=f"v{i}", name=f"v{i}") for i in range(2)]
        for i in range(2):
            nc.any.memset(v_sb[i][:, NB:NB + 2], 0.0)
        p_acc = [mp.tile([P, NB], F16, tag=f"p{i}", name=f"p{i}") for i in range(2)]

        # ============ phase 1: QK token-major (+gram+sq), V =================
        with tc.tile_pool(name=f"qs{l}", bufs=4) as qs, \
             tc.tile_pool(name=f"il{l}", bufs=1) as ilp, \
             tc.tile_pool(name=f"qps{l}", bufs=2, space="PSUM") as qps, \
             tc.tile_pool(name=f"gps{l}", bufs=1, space="PSUM") as gps:
            il_sb = [ilp.tile([P, NB], F16, tag=f"il{i}", name=f"il{i}")
                     for i in range(2)]
            for i in range(2):
                nc.sync.dma_start(il_sb[i][:], il_d.ap()[i])

            vbase, vtot = rv0 * WG, (rv1 - rv0) * WG
            for mt in range(2):
                for (o, n) in _ntiles(vtot, 512):
                    v_ps = qps.tile([P, 512], F32, tag="vps", name="vps")
                    for kt in range(2):
                        nc.tensor.matmul(
                            v_ps[:, :n],
                            w["wv"][kt][mt][:],
                            xa[kt][:, vbase + o: vbase + o + n],
                            start=(kt == 0), stop=(kt == 1))
                    nc.vector.tensor_tensor(
                        v_sb[mt][:, vbase + o: vbase + o + n],
                        v_ps[:, :n],
                        il_sb[mt][:, vbase + o: vbase + o + n], ALU.mult)

            g_ps = [gps.tile([P, 256], F32, tag=f"g{mt}", name=f"g{mt}") for mt in range(2)]
            sq_ps = gps.tile([P, 512], F32, tag="sq", name="sq")
            toks = _ntiles((rq1 - rq0) * WG, P)
            qbase = rq0 * WG
            ntk = len(toks)
            for ti, (o, m) in enumerate(toks):
                qk_ps = qps.tile([P, 512], F32, tag="qkps", name="qkps")
                for kt in range(2):
                    nc.tensor.matmul(
                        qk_ps[:m, :],
                        xa[kt][:, qbase + o: qbase + o + m],
                        w["wqk"][kt][:],
                        start=(kt == 0), stop=(kt == 1))
                qk_h = qs.tile([P, 512], F16, tag="qkh", name="qkh")
                nc.scalar.activation(qk_h[:m, :], qk_ps[:m, :], AF.Copy)
                qk_sq = qs.tile([P, 512], F32R, tag="qksq", name="qksq")
                nc.scalar.activation(qk_sq[:m, :], qk_ps[:m, :], AF.Square)
                for mt in range(2):
                    nc.tensor.matmul(
                        g_ps[mt][:, :],
                        qk_h[:m, 256 + P * mt: 256 + P * (mt + 1)],
                        qk_h[:m, 0:256],
                        start=(ti == 0), stop=(ti == ntk - 1))
                nc.tensor.matmul(
                    sq_ps[0:1, :],
                    ones_r[:m, 0:1],
                    qk_sq[:m, :],
                    start=(ti == 0), stop=(ti == ntk - 1))

            for mt in range(2):
                g_st = qs.tile([P, 256], F32, tag="gst", name="gst")
                nc.vector.tensor_copy(g_st[:], g_ps[mt][:, :])
                nc.sync.dma_start(cc_in[P * mt: P * (mt + 1), 0:256], g_st[:])
            sq_st = qs.tile([P, 512], F32, tag="sqst", name="sqst")
            nc.vector.tensor_copy(sq_st[0:1, :], sq_ps[0:1, :])
            nc.sync.dma_start(cc_in[:, 256:257], sq_st[0:1, 0:256])
            nc.sync.dma_start(cc_in[:, 257:258], sq_st[0:1, 256:512])
            nc.gpsimd.collective_compute(
                "AllReduce", ALU.add, replica_groups=GROUPS,
                ins=[cc_in.opt()], outs=[cc_out.opt()])

        dbg("v", v_sb[0][:], (P, NB))

        wload("pos")
        # ============ phase 3: positional convs (ct0 on PE diag-matmuls,
        # ct1 on DVE products+adds; overlaps the allreduce) ================
        with tc.tile_pool(name=f"pos{l}", bufs=1) as cp, \
             tc.tile_pool(name=f"posa{l}", bufs=2) as ca, \
             tc.tile_pool(name=f"psps{l}", bufs=2, space="PSUM") as pps:
            pg = [cp.tile([P, NB + 2], F16, tag=f"pg{i}", name=f"pg{i}") for i in range(2)]
            vs = cp.tile([P, NB + 2], F16, tag="vs1", name="vs1")

            # --- ct 0: PE diagonal taps ---
            s, e = rp10 * WG + 1, rp11 * WG
            for (o, n) in _ntiles(e - s, 512):
                ps1 = pps.tile([P, 512], F32, tag="ps1", name="ps1")
                _conv9_pe(nc, ps1[:, :n], w["posdiag"][0][0], v_sb[0][:],
                          0, s + o, n)
                nc.scalar.activation(pg[0][:, s + o: s + o + n], ps1[:, :n],
                                     AF.Gelu)
            _zero_guards(nc, pg[0][:, 0:NB], rp10, rp11)
            _mask_rows(nc, pg[0][:, 0:NB], rmask, rp10, rp11)
            nc.any.memset(pg[0][:, NB:NB + 2], 0.0)
            s2, e2 = ry0 * WG, ry1 * WG
            for (o, n) in _ntiles(e2 - s2, 512):
                ps2 = pps.tile([P, 512], F32, tag="ps2", name="ps2")
                _conv9_pe(nc, ps2[:, :n], w["posdiag"][1][0], pg[0][:],
                          0, s2 + o, n)
                nc.scalar.activation(p_acc[0][:, s2 + o: s2 + o + n],
                                      ps2[:, :n], AF.Copy)

            # --- ct 1: DVE ---
            _shift1(nc, vs[:], v_sb[1][:, 0:NB], NB)
            for (co, cn) in _ntiles(rp11 - rp10, POS_CH):
                a, b = rp10 + co, rp10 + co + cn
                acc = _conv9(nc, ca, "cv", None, v_sb[1][:], vs[:],
                             w["posw"][0][1], a, b)
                nc.scalar.activation(
                    pg[1][:, a * WG: b * WG], acc, AF.Gelu)
            _zero_guards(nc, pg[1][:, 0:NB], rp10, rp11)
            _mask_rows(nc, pg[1][:, 0:NB], rmask, rp10, rp11)
            nc.any.memset(pg[1][:, NB:NB + 2], 0.0)
            for (o, n) in _ntiles(e2 - s2, 512):
                ps3 = pps.tile([P, 512], F32, tag="ps2", name="ps2")
                _conv9_pe(nc, ps3[:, :n], w["posdiag"][1][1], pg[1][:],
                          0, s2 + o, n)
                nc.scalar.activation(p_acc[1][:, s2 + o: s2 + o + n],
                                     ps3[:, :n], AF.Copy)

        dbg("p", p_acc[0][:], (P, NB))

        # ============ phase 2: attention epilogue (after allreduce) =========
        with tc.tile_pool(name=f"att{l}", bufs=1) as ap, \
             tc.tile_pool(name=f"aps{l}", bufs=1, space="PSUM") as aps:
            g_sb = [ap.tile([P, 256], F32, tag=f"gs{mt}", name=f"gs{mt}") for mt in range(2)]
            for mt in range(2):
                nc.sync.dma_start(g_sb[mt][:],
                                  cc_out[P * mt: P * (mt + 1), 0:256])
            sq_row = ap.tile([P, 256], F32R, tag="sqrow", name="sqrow")
            nc.sync.dma_start(sq_row[0:1, :].bitcast(F32), cc_out[:, 256:257])
            sqk = [ap.tile([P, 1], F32, tag=f"sqk{mt}", name=f"sqk{mt}") for mt in range(2)]
            for mt in range(2):
                nc.sync.dma_start(sqk[mt][:],
                                  cc_out[P * mt: P * (mt + 1), 257:258])

            def rsq(dst):
                with nc.allow_low_precision(reason="attn scale factors"):
                    nc.scalar.activation(dst, dst, AF.Sqrt)
                    nc.vector.tensor_scalar_max(dst, dst, 1e-12)
                    nc.vector.reciprocal(dst, dst)

            rsq(sq_row[0:1, :])
            for mt in range(2):
                rsq(sqk[mt][:])
                nc.vector.tensor_mul(sqk[mt][:], sqk[mt][:], w["resc"][mt][:])
            bc_ps = aps.tile([P, 256], F32, tag="bcps", name="bcps")
            nc.tensor.matmul(bc_ps[:, :], ones_r[0:1, :],
                             sq_row[0:1, :], start=True, stop=True)
            bc_sb = ap.tile([P, 256], F32, tag="bcsb", name="bcsb")
            nc.vector.tensor_copy(bc_sb[:], bc_ps[:])

            z_sb = ap.tile([P, 2, 32], F32, tag="z", name="z")
            for h in range(HEADS):
                ct, po = h // 4, 32 * (h % 4)
                nc.vector.scalar_tensor_tensor(
                    z_sb[po:po + 32, ct, :],
                    g_sb[ct][po:po + 32, 32 * h: 32 * h + 32],
                    sqk[ct][po:po + 32, :],
                    bc_sb[po:po + 32, 32 * h: 32 * h + 32],
                    ALU.mult, ALU.mult)
            att16 = ap.tile([P, 2, 32], F16, tag="att16", name="att16")
            nmax = ap.tile([P, 2], F32, tag="nmax", name="nmax")
            ssum = ap.tile([P, 2], F32, tag="ssum", name="ssum")
            esb = ap.tile([P, 2, 32], F32, tag="esb", name="esb")
            for s in range(2):
                nc.vector.tensor_reduce(nmax[:, s:s + 1], z_sb[:, s, :],
                                        mybir.AxisListType.X, ALU.max,
                                        negate=True)
                nc.scalar.activation(esb[:, s, :], z_sb[:, s, :], AF.Exp,
                                     bias=nmax[:, s:s + 1],
                                     accum_out=ssum[:, s:s + 1])
            nc.vector.reciprocal(ssum[:], ssum[:])
            for s in range(2):
                nc.vector.tensor_scalar_mul(att16[:, s, :], esb[:, s, :],
                                            ssum[:, s:s + 1])
            for h in range(HEADS):
                ct, po = h // 4, 32 * (h % 4)
                nc.vector.transpose(attnT[po:po + 32, ct, :],
                                    att16[po:po + 32, ct, :])
            dbg("attnT", attnT[:], (P, 2, 32))

        wload("oy")
        # ============ phase 4: o = attnT @ v, proj, y = x + out_c + p =======
        with tc.tile_pool(name=f"oy{l}", bufs=3) as osp, \
             tc.tile_pool(name=f"oyps{l}", bufs=2, space="PSUM") as ops:
            ybase, ytot = ry0 * WG, (ry1 - ry0) * WG
            for (o, n) in _ntiles(ytot, 512):
                o_h = []
                for ct in range(2):
                    o_ps = ops.tile([P, 512], F32, tag=f"ops{ct}", name=f"ops{ct}")
                    for r in range(4):
                        po = 32 * r
                        nc.tensor.matmul(
                            o_ps[po:po + 32, :n],
                            attnT[po:po + 32, ct, :],
                            v_sb[ct][po:po + 32, ybase + o: ybase + o + n],
                            start=True, stop=True, tile_position=(po, po))
                    oh = osp.tile([P, 512], F16, tag=f"oh{ct}", name=f"oh{ct}")
                    nc.scalar.activation(oh[:, :n], o_ps[:, :n], AF.Copy)
                    o_h.append(oh)
                for mt in range(2):
                    pr_ps = ops.tile([P, 512], F32, tag=f"prps{mt}", name=f"prps{mt}")
                    for kt in range(2):
                        nc.tensor.matmul(pr_ps[:, :n], w["wpr"][kt][mt][:],
                                         o_h[kt][:, :n],
                                         start=(kt == 0), stop=(kt == 1))
                    y1 = osp.tile([P, 512], F32, tag=f"y1{mt}", name=f"y1{mt}")
                    nc.vector.scalar_tensor_tensor(
                        y1[:, :n], pr_ps[:, :n], w["bpj"][mt][:],
                        p_acc[mt][:, ybase + o: ybase + o + n],
                        ALU.add, ALU.add)
                    nc.gpsimd.tensor_tensor(
                        xb[mt][:, ybase + o: ybase + o + n],
                        y1[:, :n],
                        xa[mt][:, ybase + o: ybase + o + n],
                        ALU.add)

        for ct in range(2):
            _mask_rows(nc, xb[ct][:], rmask, ry0, ry1)

        dbg("y", xb[0][:], (P, NB))

    wload("ff")
    # ============ phase 5: LN + FF (chunked over rows) ======================
    with tc.tile_pool(name=f"ff{l}", bufs=1) as fp, \
         tc.tile_pool(name=f"ffs{l}", bufs=1) as fs, \
         tc.tile_pool(name=f"ffps{l}", bufs=2, space="PSUM") as fps, \
         tc.tile_pool(name=f"stps{l}", bufs=1, space="PSUM") as sps:
        for (co, cn) in _ntiles(ry1 - ry0, FF_CH):
            a, b = ry0 + co, ry0 + co + cn
            w0, w1 = max(a - 1, ry0), min(b + 1, ry1)
            c0, c1 = max(a, rdw0), min(b, rdw1)
            wlen = (w1 - w0) * WG
            # --- LN stats over channels (broadcast via ones matmul) ---
            ysq = [fs.tile([P, wlen], F16, tag=f"ysq{ct}", name=f"ysq{ct}") for ct in range(2)]
            for ct in range(2):
                nc.scalar.activation(ysq[ct][:],
                                     xb[ct][:, w0 * WG: w0 * WG + wlen],
                                     AF.Square)
            # stats + rs + xln per 512-subtile (keeps PSUM lifetimes short)
            rs = fs.tile([P, wlen], F32, tag="rs", name="rs")
            rs16 = fs.tile([P, wlen], F16, tag="rs16", name="rs16")
            xln = [fs.tile([P, wlen], F16, tag=f"xln{ct}", name=f"xln{ct}") for ct in range(2)]
            for (o, n) in _ntiles(wlen, 512):
                ssum = sps.tile([P, 512], F32, tag="ssum", name="ssum", bufs=2)
                ssq = sps.tile([P, 512], F32, tag="ssq", name="ssq", bufs=2)
                for ct in range(2):
                    nc.tensor.matmul(ssum[:, :n], ones_h[:, :],
                                     xb[ct][:, w0 * WG + o: w0 * WG + o + n],
                                     start=(ct == 0), stop=(ct == 1))
                    nc.tensor.matmul(ssq[:, :n], ones_h[:, :],
                                     ysq[ct][:, o:o + n],
                                     start=(ct == 0), stop=(ct == 1))
                mu2 = fs.tile([P, 512], F32, tag="mu2", name="mu2", bufs=2)
                nc.scalar.activation(mu2[:, :n], ssum[:, :n], AF.Square,
                                     scale=cinv_t[:])
                # rs <- ln(var + eps); exponentiated once per chunk below
                nc.vector.scalar_tensor_tensor(rs[:, o:o + n], ssq[:, :n],
                                               1.0 / C, mu2[:, :n],
                                               ALU.mult, ALU.subtract)
                nc.scalar.activation(rs16[:, o:o + n], rs[:, o:o + n],
                                     AF.Abs_reciprocal_sqrt, bias=eps_t[:])
                for ct in range(2):
                    d = fs.tile([P, 512], F16, tag=f"d{ct}", name=f"d{ct}", bufs=2)
                    nc.vector.scalar_tensor_tensor(
                        d[:, :n], ssum[:, :n], -1.0 / C,
                        xb[ct][:, w0 * WG + o: w0 * WG + o + n],
                        ALU.mult, ALU.add)
                    nc.vector.tensor_scalar_mul(xln[ct][:, o:o + n], d[:, :n],
                                                w["lng"][ct][:])
            for ct in range(2):
                nc.vector.tensor_tensor(xln[ct][:], xln[ct][:], rs16[:],
                                        ALU.mult)
            if co == 0:
                dbg("xln", xln[0][:], (P, wlen))
            # --- ff1 + gelu -> t1 ---
            t1 = [fs.tile([P, wlen + 2], F16, tag=f"t1{mt}", name=f"t1{mt}") for mt in range(8)]
            t1s = [fs.tile([P, wlen + 2], F16, tag=f"t1s{mt}", name=f"t1s{mt}")
                   if mt in FFDW_DVE_MT else None for mt in range(8)]
            for mt in range(8):
                for (o, n) in _ntiles(wlen, 512):
                    f1_ps = fps.tile([P, 512], F32, tag="ffps", name="ffps")
                    for kt in range(2):
                        nc.tensor.matmul(f1_ps[:, :n], w["f1t"][kt][mt][:],
                                         xln[kt][:, o:o + n],
                                         start=(kt == 0), stop=(kt == 1))
                    nc.scalar.activation(t1[mt][:, o:o + n], f1_ps[:, :n],
                                         AF.Gelu, bias=w["b1"][mt][:])
                _zero_guards(nc, t1[mt][:, 0:wlen], 0, w1 - w0)
                nc.any.memset(t1[mt][:, wlen:wlen + 2], 0.0)
                if mt in FFDW_DVE_MT:
                    _shift1(nc, t1s[mt][:], t1[mt][:, 0:wlen], wlen)
            # --- ffdw (PE diagonal taps) + gelu -> t2 ---
            t2 = [fs.tile([P, (c1 - c0) * WG], F16, tag=f"t2{mt}", name=f"t2{mt}", bufs=2)
                  for mt in range(8)]
            for mt in range(8):
                if mt in FFDW_DVE_MT:
                    acc = _conv9(nc, fs, "dw", None, t1[mt][:, 0:wlen],
                                 t1s[mt][:], w["dww"][mt], c0 - w0, c1 - w0)
                    nc.scalar.activation(t2[mt][:], acc, AF.Gelu)
                else:
                    base = (c0 - w0) * WG
                    for (o, n) in _ntiles((c1 - c0) * WG - 1, 512):
                        dw_ps = fps.tile([P, 512], F32, tag="dwps", name="dwps")
                        _conv9_pe(nc, dw_ps[:, :n], w["dwdiag"][mt],
                                  t1[mt][:], c0 - w0, o + 1, n)
                        nc.scalar.activation(t2[mt][:, 1 + o:1 + o + n],
                                             dw_ps[:, :n], AF.Gelu)
                    nc.any.memset(t2[mt][:, 0:1], 0.0)
            # --- ff3 + residual -> x2 (= xa), or final output ---
            for mt in range(2):
                for (o, n) in _ntiles((c1 - c0) * WG, 512):
                    f3_ps = fps.tile([P, 512], F32, tag="ffps", name="ffps")
                    for kt in range(8):
                        nc.tensor.matmul(f3_ps[:, :n], w["f3t"][kt][mt][:],
                                         t2[kt][:, o:o + n],
                                         start=(kt == 0), stop=(kt == 7))
                    nc.vector.tensor_tensor(
                        xa[mt][:, c0 * WG + o: c0 * WG + o + n],
                        f3_ps[:, :n],
                        xb[mt][:, c0 * WG + o: c0 * WG + o + n], ALU.add)
            for mt in range(2):
                _zero_guards(nc, xa[mt][:], c0, c1)

    if out_d is not None:
        with tc.tile_pool(name="outp", bufs=1) as op_:
            for ct in range(2):
                o32 = op_.tile([P, 32 * WG], F32, tag=f"o32{ct}", name=f"o32{ct}")
                nc.scalar.activation(o32[:], xa[ct][:, 6 * WG: 38 * WG], AF.Copy)
                nc.sync.dma_start(out_d.ap()[ct], _r3(o32[:])[:, :, 1:129])
    else:
        dbg("x2", xa[0][:], (P, NB))


# ======================== host side =========================================

_CACHE = {}


def _prep_shards(x, illu_fea, Wq, Wk, Wv, rescale, Wproj, bproj, pos1, pos2,
                 ln_g, ln_b, ff1, ffdw, ff3):
    f16 = ml_dtypes.float16 if hasattr(ml_dtypes, "float16") else np.float16

    def pad_spatial(t):  # (B,C,H,W) -> per-core [2, 128, HB, WG]
        out = []
        for core in range(N_CORES):
            bb, ss = core // 4, core % 4
            r0 = 32 * ss - HALO
            buf = np.zeros((C, HB, WG), np.float32)
            lo, hi = max(r0, 0), min(r0 + HB, H)
            buf[:, lo - r0: hi - r0, 1:129] = t[bb, :, lo:hi, :]
            out.append(buf.reshape(2, P, HB, WG))
        return out

    xs = pad_spatial(np.asarray(x, np.float32))
    ils = pad_spatial(np.asarray(illu_fea, np.float32))

    wqk = np.stack([np.concatenate([Wq[l], Wk[l]], axis=1) for l in range(LYR)])
    wqk = wqk.reshape(LYR, 2, P, 512)
    wv = np.stack([Wv[l].reshape(2, P, 2, P).transpose(0, 2, 1, 3)
                   for l in range(LYR)])
    wpr = np.stack([Wproj[l].reshape(2, P, 2, P).transpose(0, 2, 1, 3)
                    for l in range(LYR)])
    f1 = np.stack([ff1[l, :, :, 0, 0].T.reshape(2, P, 8, P).transpose(0, 2, 1, 3)
                   for l in range(LYR)])
    f3 = np.stack([ff3[l, :, :, 0, 0].T.reshape(8, P, 2, P).transpose(0, 2, 1, 3)
                   for l in range(LYR)])
    posw = np.stack([np.stack([p[l, :, 0].reshape(C, 9).reshape(2, P, 9)
                               for p in (pos1, pos2)]) for l in range(LYR)])
    dww = np.stack([ffdw[l, :, 0].reshape(FC, 9).reshape(8, P, 9)
                    for l in range(LYR)])
    dwdg = np.zeros((LYR, 8, 9, P, P), np.float32)
    psdg = np.zeros((LYR, 2, 2, 9, P, P), np.float32)
    ii = np.arange(P)
    for l in range(LYR):
        for mt in range(8):
            for t in range(9):
                dwdg[l, mt, t, ii, ii] = dww[l, mt, :, t]
        for cv in range(2):
            for ct in range(2):
                for t in range(9):
                    psdg[l, cv, ct, t, ii, ii] = posw[l, cv, ct, :, t]
    lng = np.asarray(ln_g, np.float32).reshape(LYR, 2, P, 1)
    b1 = np.stack([(ff1[l, :, :, 0, 0] @ ln_b[l]).reshape(8, P, 1)
                   for l in range(LYR)])
    bpj = np.asarray(bproj, np.float32).reshape(LYR, 2, P, 1)
    resc = np.stack([np.repeat(rescale[l, :, 0, 0], D).reshape(2, P, 1)
                     for l in range(LYR)])

    const = {
        "wqk": wqk.astype(f16), "wv": wv.astype(f16), "wpr": wpr.astype(f16),
        "f1t": f1.astype(f16), "f3t": f3.astype(f16),
        "posw": posw.astype(np.float32), "dww": dww.astype(np.float32),
        "dwdg": dwdg.astype(f16), "psdg": psdg.astype(f16),
        "lng": lng, "b1": b1.astype(np.float32), "bpj": bpj,
        "resc": resc.astype(np.float32),
    }
    in_maps = []
    for core in range(N_CORES):
        m = dict(const)
        ss = core % 4
        r0 = 32 * ss - HALO
        rmv = np.zeros((P, HB), np.float32)
        for r in range(HB):
            rmv[:, r] = 1.0 if 0 <= r0 + r < H else 0.0
        m["rmask"] = rmv.astype(np.float32)
        m["xh"] = xs[core].reshape(2, P, NB).astype(f16)
        m["il"] = ils[core].reshape(2, P, NB).astype(f16)
        in_maps.append(m)
    return in_maps


def _get_nc():
    if "nc" not in _CACHE:
        _CACHE["nc"], _CACHE["dbg"] = build()
    return _CACHE["nc"]


def run(in_maps, trace=False):
    nc = _get_nc()
    return bass_utils.run_bass_kernel_spmd(
        nc, in_maps, core_ids=list(range(N_CORES)), trace=trace)


def kernel(**inputs):
    in_maps = _prep_shards(**{k: np.asarray(v) for k, v in inputs.items()})
    res = run(in_maps)
    out = np.zeros((B, C, H, W), np.float32)
    for core in range(N_CORES):
        bb, ss = core // 4, core % 4
        o = res.results[core]["out"]  # [2, 128, 32, 128]
        out[bb, :, 32 * ss: 32 * ss + 32, :] = o.reshape(C, 32, W)
    return out

